# revision 19
# baseline (speedup 1.0000x reference)
"""Bahdanau attention kernel for Trainium2 (Bass/Tile), data-parallel over batch.

Problem (full shapes):
    encoder_output   [S=2048, B=16, H=1024] f32
    last_decoder_state [2, 1, B, H] f32   (only [0,0] used -> state [B, H])
    W [H, H], b [H]
    energy = state @ W.T + b                  [B, H]
    scores = einsum('sbh,bh->sb', enc, energy) [S, B]
    out    = softmax(scores, axis=0)[None, None]  [1, 1, S, B]

Sharding: batch split across 8 cores (2 batches each); W/b replicated.
Softmax is over S which is fully resident per core -> no collectives.

The kernel is DMA-bound (memory regime): per core it must stream its
16.8 MB enc slice + 4 MB W per rep in f32.  Both are cast to fp16 on the
host (validated: final rel err 6.8e-3 vs the 2e-2 gate; bf16 fails at
5.4e-2), halving HBM traffic to ~10.4 MB -> ~29 us roofline at 358 GB/s.

Per-core device program (all host-side layouts are DMA-natural, every
descriptor is a contiguous 2-8 KB run):
    energy^T[b, j] = sum_i state[b,i] W[j,i]           (PE; the 2-column
        state is the stationary operand so LDWEIGHTS is ~free and W
        streams through the moving path at 1 col/cycle — measured 64
        W-stationary matmuls cost ~9.4us/rep, LDWEIGHTS-dominated)
    energy[j, b] via 8 PE transposes (2x2 identity) + bias on DVE
    scores[b, s] = sum_h energy[h, b] enc[b, h, s]     (PE, fp16, f32 PSUM
        accum; the two batches run concurrently in separate 32-col groups
        - tile_position auto-derives from the out partition 0/32)
    probs = softmax over s                             (joint [33,*] ops:
        both batches (partitions 0/32) in one instruction per stage)

All input DMAs ride one HWDGE ring (sync engine) in FIFO order, W first
so energy is ready before the enc tiles it gates; the output DMA rides
the scalar ring so a rep's tail store can never block the next rep's
input stream.

`reps`/`dynamic` exist only for benchmarking: they repeat the body inside
one NEFF so HW time can be measured through a high-latency dispatch path.
The dynamic form amortizes For_i's all-engine barrier over UNROLL body
copies per iteration, and the tile pools double-buffer across body copies
so reps pipeline (steady-state throughput, DMA-limited). kernel() always
uses reps=1.
"""

import numpy as np

S, B, H = 2048, 16, 1024
NCORES = 8
BL = B // NCORES  # 2 batches per core
P = 128           # partitions
HT = H // P       # 8 h-tiles
SCW = 512         # matmul moving-operand chunk (one PSUM bank of f32)
SC = S // SCW     # 4 seq chunks
UNROLL = 8        # body copies per For_i iteration in dynamic bench mode
ENC_CHUNK = 1     # h-tiles per enc dma_start
ENC_BUFS = 3
W_BUFS = 3
ENERGY_MODE = "sstat"  # "wstat": W stationary (64 matmuls); "sstat": state
                       # stationary (16 wide matmuls + 8 PE transposes)

_cached = {}


def _build_nc(reps=1, dynamic=False):
    import concourse.bacc as bacc
    import concourse.bass as bass
    import concourse.tile as tile
    from concourse import mybir

    f16 = mybir.dt.float16
    f32 = mybir.dt.float32
    nc = bacc.Bacc("TRN2", target_bir_lowering=False, debug=False, num_devices=NCORES)

    # host-prepped layouts (see prep_in_maps):
    # enc_t[ht, p, bl, s] = enc[s, b0+bl, ht*128+p]          fp16
    # wtb[jt, p, it, j]   = W[jt*128+j, it*128+p]            fp16
    # st_in[p, it, bl]    = state[b0+bl, it*128+p]           fp16
    # bt_in[p, jt]        = bias[jt*128+p]                   f32
    enc_t = nc.dram_tensor("enc_t", [HT, P, BL, S], f16, kind="ExternalInput").ap()
    wtb = nc.dram_tensor("wtb", [HT, P, HT, P], f16, kind="ExternalInput").ap()
    wit = nc.dram_tensor("wit", [HT, P, H], f16, kind="ExternalInput").ap()
    st_in = nc.dram_tensor("st_in", [P, HT, BL], f16, kind="ExternalInput").ap()
    bt_in = nc.dram_tensor("bt_in", [P, HT], f32, kind="ExternalInput").ap()
    id_in = nc.dram_tensor("id_in", [BL, BL], f16, kind="ExternalInput").ap()
    probs = nc.dram_tensor("probs", [BL, S], f32, kind="ExternalOutput").ap()

    with tile.TileContext(nc) as tc:
        sstat = ENERGY_MODE == "sstat"
        with (
            tc.tile_pool(name="consts", bufs=2) as consts,
            tc.tile_pool(name="wpool", bufs=W_BUFS) as wpool,
            tc.tile_pool(name="encpool", bufs=ENC_BUFS) as encpool,
            tc.tile_pool(name="pe_ps", bufs=1 if sstat else 2,
                         space=bass.MemorySpace.PSUM) as pe_pool,
            tc.tile_pool(name="sc_ps", bufs=1, space=bass.MemorySpace.PSUM) as ps_pool,
            tc.tile_pool(name="tp_ps", bufs=2, space=bass.MemorySpace.PSUM) as tp_pool,
            tc.tile_pool(name="spool", bufs=2) as spool,
        ):

            def emit_rep():
                st = consts.tile([P, HT, BL], f16)
                nc.sync.dma_start(out=st[:], in_=st_in)
                bt = consts.tile([P, HT], f32)
                nc.sync.dma_start(out=bt[:], in_=bt_in)

                # input stream, one FIFO ring: W early/interleaved so energy
                # is ready long before the enc tiles it gates
                wts = [None] * HT
                wgs = [None] * 2
                ets = [None] * HT

                def load_w(jt):
                    wt = wpool.tile([P, HT, P], f16)
                    nc.sync.dma_start(out=wt[:], in_=wtb[jt])
                    wts[jt] = wt

                def load_wg(g):
                    # sstat: 1 MB group of 4 W i-tiles [p, it, j]
                    wg = wpool.tile([P, 4, H], f16)
                    nc.sync.dma_start(
                        out=wg[:],
                        in_=wit[4 * g:4 * (g + 1)].rearrange("t p j -> p t j"),
                    )
                    wgs[g] = wg

                def load_e(h0):
                    # one transfer covering h-tiles [h0, h0+ENC_CHUNK)
                    et = encpool.tile([P, ENC_CHUNK, BL, S], f16)
                    nc.sync.dma_start(
                        out=et[:],
                        in_=enc_t[h0:h0 + ENC_CHUNK].rearrange("t p b s -> p t b s"),
                    )
                    for k in range(ENC_CHUNK):
                        ets[h0 + k] = (et, k)

                eloads = [h0 for h0 in range(0, HT, ENC_CHUNK)]
                if sstat:
                    load_wg(0)
                    load_wg(1)
                    for h0 in eloads:
                        load_e(h0)
                else:
                    wloads = list(range(HT))
                    order = []
                    for h0 in eloads:
                        while wloads and len(order) % 3 != 2:
                            order.append(("w", wloads.pop(0)))
                        order.append(("e", h0))
                    for it_, idx in order:
                        (load_w if it_ == "w" else load_e)(idx)
                    for jt in wloads:
                        load_w(jt)

                # energy[j % 128, jt, b] = sum_i W[j,i] state[b,i] + bias[j]
                energy = consts.tile([P, HT, BL], f16)
                if sstat:
                    # energy_T[b, j] accumulated over i-tiles; state is the
                    # (2-column) stationary operand so LDWEIGHTS is ~free and
                    # W streams through the moving path at 1 col/cycle
                    et_ps = pe_pool.tile([BL, H], f32)
                    for it in range(HT):
                        for jh in range(2):
                            nc.tensor.matmul(
                                et_ps[:, jh * SCW:(jh + 1) * SCW],
                                st[:, it, :],                            # lhsT [i, b]
                                wgs[it // 4][:, it % 4, jh * SCW:(jh + 1) * SCW],
                                start=(it == 0),
                                stop=(it == HT - 1),
                            )
                    esb = consts.tile([BL, H], f16)
                    nc.scalar.activation(
                        out=esb[:], in_=et_ps[:],
                        func=mybir.ActivationFunctionType.Identity,
                        bias=0.0, scale=1.0,
                    )
                    id2 = consts.tile([BL, BL], f16)
                    nc.sync.dma_start(out=id2[:], in_=id_in)
                    for jt in range(HT):
                        tp = tp_pool.tile([P, BL], f16)
                        nc.tensor.transpose(
                            tp[:], esb[:, jt * P:(jt + 1) * P], id2[:]
                        )
                        nc.vector.tensor_scalar_add(
                            out=energy[:, jt, :], in0=tp[:],
                            scalar1=bt[:, jt:jt + 1],
                        )
                else:
                    for jt in range(HT):
                        pe = pe_pool.tile([P, BL], f32)
                        for it in range(HT):
                            nc.tensor.matmul(
                                pe[:],
                                wts[jt][:, it, :],  # lhsT [i, j]
                                st[:, it, :],       # rhs  [i, b]
                                start=(it == 0),
                                stop=(it == HT - 1),
                            )
                        nc.vector.tensor_scalar_add(
                            out=energy[:, jt, :], in0=pe[:], scalar1=bt[:, jt:jt + 1]
                        )

                # scores psum: b=0 on partition 0, b=1 on partition 32; the
                # two batches run concurrently in separate 32-col groups
                ps = ps_pool.tile([33, S], f32)
                for ht in range(HT):
                    et, k = ets[ht]
                    for sc in range(SC):
                        for b in range(BL):
                            nc.tensor.matmul(
                                ps[32 * b:32 * b + 1, sc * SCW:(sc + 1) * SCW],
                                energy[:, ht, b:b + 1],                    # lhsT [h, 1]
                                et[:, k, b, sc * SCW:(sc + 1) * SCW],      # rhs [h, s]
                                start=(ht == 0),
                                stop=(ht == HT - 1),
                                tile_position=(0, 32 * b),
                            )

                # softmax over s (free dim); both batches (partitions 0 and
                # 32) processed jointly in one instruction per stage
                prob_sb = spool.tile([33, S], f32)
                nmax = spool.tile([33, 1], f32)
                ssum = spool.tile([33, 1], f32)
                rinv = spool.tile([33, 1], f32)
                nc.vector.reduce_max(
                    nmax[:], ps[:], axis=mybir.AxisListType.X, negate=True
                )
                nc.scalar.activation(
                    out=prob_sb[:],
                    in_=ps[:],
                    func=mybir.ActivationFunctionType.Exp,
                    bias=nmax[:],
                    scale=1.0,
                    accum_out=ssum[:],
                )
                nc.vector.reciprocal(rinv[:], ssum[:])
                nc.vector.tensor_scalar_mul(
                    out=prob_sb[:], in0=prob_sb[:], scalar1=rinv[:]
                )
                # output on the scalar ring: a tail store must never block
                # the next rep's input stream on the sync ring
                for b in range(BL):
                    nc.scalar.dma_start(
                        out=probs[b:b + 1, :], in_=prob_sb[32 * b:32 * b + 1, :]
                    )

            if dynamic and reps > 1:
                assert reps % UNROLL == 0, (reps, UNROLL)
                with tc.For_i(0, reps // UNROLL, 1):
                    for _u in range(UNROLL):
                        emit_rep()
            else:
                for _rep in range(reps):
                    emit_rep()

    nc.compile()
    return nc


def get_nc(reps=1, dynamic=False):
    key = ("nc", reps, dynamic, UNROLL, ENC_CHUNK, ENC_BUFS, W_BUFS, ENERGY_MODE)
    if key not in _cached:
        _cached[key] = _build_nc(reps, dynamic)
    return _cached[key]


def prep_in_maps(encoder_output, last_decoder_state, W, b):
    enc16 = np.asarray(encoder_output, dtype=np.float32).astype(np.float16)  # [S,B,H]
    state = np.asarray(last_decoder_state, dtype=np.float32)[0, 0]           # [B,H]
    W32 = np.asarray(W, dtype=np.float32)
    # wtb[jt, p, it, j] = W[jt*128+j, it*128+p]
    W16 = W32.astype(np.float16)
    wtb = np.ascontiguousarray(W16.reshape(HT, P, HT, P).transpose(0, 3, 2, 1))
    wit = np.ascontiguousarray(W16.T).reshape(HT, P, H)
    bias = np.asarray(b, dtype=np.float32)
    bt = np.ascontiguousarray(bias.reshape(HT, P).T)                         # [p, jt]
    in_maps = []
    for c in range(NCORES):
        b0 = BL * c
        ec = enc16[:, b0:b0 + BL, :]                                         # [S,BL,H]
        enc_t = np.ascontiguousarray(ec.transpose(2, 1, 0)).reshape(HT, P, BL, S)
        stc = state[b0:b0 + BL, :].astype(np.float16)                        # [BL,H]
        st = np.ascontiguousarray(stc.reshape(BL, HT, P).transpose(2, 1, 0))
        in_maps.append({"enc_t": enc_t, "wtb": wtb, "wit": wit,
                        "st_in": st, "bt_in": bt,
                        "id_in": np.eye(BL, dtype=np.float16)})
    return in_maps


def assemble(results):
    out = np.empty((S, B), np.float32)
    for c in range(NCORES):
        out[:, BL * c:BL * (c + 1)] = results[c]["probs"].T
    return out[None, None]


def kernel(encoder_output, last_decoder_state, W, b):
    from concourse.bass_utils import run_bass_kernel_spmd

    nc = get_nc()
    in_maps = prep_in_maps(encoder_output, last_decoder_state, W, b)
    res = run_bass_kernel_spmd(nc, in_maps, core_ids=list(range(NCORES)))
    return assemble(res.results)


# revision 27
# speedup vs baseline: 2.8015x; 2.8015x over previous
"""Bahdanau attention kernel for Trainium2 (Bass/Tile), data-parallel over batch.

Problem (full shapes):
    encoder_output   [S=2048, B=16, H=1024] f32
    last_decoder_state [2, 1, B, H] f32   (only [0,0] used -> state [B, H])
    W [H, H], b [H]
    energy = state @ W.T + b                  [B, H]
    scores = einsum('sbh,bh->sb', enc, energy) [S, B]
    out    = softmax(scores, axis=0)[None, None]  [1, 1, S, B]

Sharding: batch split across 8 cores (2 batches each); W/b replicated.
Softmax is over S which is fully resident per core -> no collectives.

The kernel is DMA-bound (memory regime): per core it must stream its
16.8 MB enc slice + 4 MB W per rep in f32.  Both are cast to fp16 on the
host (validated: final rel err 6.8e-3 vs the 2e-2 gate; bf16 fails at
5.4e-2), halving HBM traffic to ~10.4 MB -> ~29 us roofline at 358 GB/s.

Per-core device program (all host-side layouts are DMA-natural, every
descriptor is a contiguous 2-8 KB run):
    energy^T[b, j] = sum_i state[b,i] W[j,i]           (PE; the 2-column
        state is the stationary operand so LDWEIGHTS is ~free and W
        streams through the moving path at 1 col/cycle — measured 64
        W-stationary matmuls cost ~9.4us/rep, LDWEIGHTS-dominated)
    energy[j, b] via 8 PE transposes (2x2 identity) + bias on DVE
    scores[b, s] = sum_h energy[h, b] enc[b, h, s]     (PE, fp16, f32 PSUM
        accum; the two batches run concurrently in separate 32-col groups
        - tile_position auto-derives from the out partition 0/32)
    probs = softmax over s                             (joint [33,*] ops:
        both batches (partitions 0/32) in one instruction per stage)

All input DMAs ride one HWDGE ring (sync engine) in FIFO order, W first
so energy is ready before the enc tiles it gates; the output DMA rides
the scalar ring so a rep's tail store can never block the next rep's
input stream.

`reps`/`dynamic` exist only for benchmarking: they repeat the body inside
one NEFF so HW time can be measured through a high-latency dispatch path.
The dynamic form amortizes For_i's all-engine barrier over UNROLL body
copies per iteration, and the tile pools double-buffer across body copies
so reps pipeline (steady-state throughput, DMA-limited). kernel() always
uses reps=1.
"""

import numpy as np

S, B, H = 2048, 16, 1024
NCORES = 8
BL = B // NCORES  # 2 batches per core
P = 128           # partitions
HT = H // P       # 8 h-tiles
SCW = 512         # matmul moving-operand chunk (one PSUM bank of f32)
SC = S // SCW     # 4 seq chunks
UNROLL = 64       # body copies per For_i iteration in dynamic bench mode
                  # (For_i's all-engine barrier + drain is expensive here:
                  # measured 45.1us/rep at U8, 38.1 at U16, 26.8 at U32,
                  # and with DMA_SPLIT 13.2 at U32 / 9.2 at U64)
ENC_CHUNK = 1     # h-tiles per enc dma_start
ENC_BUFS = 3
W_BUFS = 3
ENERGY_MODE = "sstat"  # "wstat": W stationary (64 matmuls); "sstat": state
                       # stationary (16 wide matmuls + 8 PE transposes)
DMA_SPLIT = True       # True: W/state/bias/id on the gpsimd SWDGE path so
                       # the sync ring only issues the 8 enc transfers
                       # (parallel descriptor-generation paths; worth ~2x —
                       # the single ring's issue serialization, not HBM
                       # bandwidth, was the binding constraint)

_cached = {}


def _build_nc(reps=1, dynamic=False):
    import concourse.bacc as bacc
    import concourse.bass as bass
    import concourse.tile as tile
    from concourse import mybir

    f16 = mybir.dt.float16
    f32 = mybir.dt.float32
    nc = bacc.Bacc("TRN2", target_bir_lowering=False, debug=False, num_devices=NCORES)

    # host-prepped layouts (see prep_in_maps):
    # enc_t[ht, p, bl, s] = enc[s, b0+bl, ht*128+p]          fp16
    # wtb[jt, p, it, j]   = W[jt*128+j, it*128+p]            fp16
    # st_in[p, it, bl]    = state[b0+bl, it*128+p]           fp16
    # bt_in[p, jt]        = bias[jt*128+p]                   f32
    enc_t = nc.dram_tensor("enc_t", [HT, P, BL, S], f16, kind="ExternalInput").ap()
    wtb = nc.dram_tensor("wtb", [HT, P, HT, P], f16, kind="ExternalInput").ap()
    wit = nc.dram_tensor("wit", [HT, P, H], f16, kind="ExternalInput").ap()
    st_in = nc.dram_tensor("st_in", [P, HT, BL], f16, kind="ExternalInput").ap()
    bt_in = nc.dram_tensor("bt_in", [P, HT], f32, kind="ExternalInput").ap()
    id_in = nc.dram_tensor("id_in", [BL, BL], f16, kind="ExternalInput").ap()
    probs = nc.dram_tensor("probs", [BL, S], f32, kind="ExternalOutput").ap()

    with tile.TileContext(nc) as tc:
        sstat = ENERGY_MODE == "sstat"
        with (
            tc.tile_pool(name="consts", bufs=2) as consts,
            tc.tile_pool(name="wpool", bufs=W_BUFS) as wpool,
            tc.tile_pool(name="encpool", bufs=ENC_BUFS) as encpool,
            tc.tile_pool(name="pe_ps", bufs=1 if sstat else 2,
                         space=bass.MemorySpace.PSUM) as pe_pool,
            tc.tile_pool(name="sc_ps", bufs=1, space=bass.MemorySpace.PSUM) as ps_pool,
            tc.tile_pool(name="tp_ps", bufs=2, space=bass.MemorySpace.PSUM) as tp_pool,
            tc.tile_pool(name="spool", bufs=2) as spool,
        ):

            weng = nc.gpsimd if DMA_SPLIT else nc.sync

            def emit_rep():
                st = consts.tile([P, HT, BL], f16)
                weng.dma_start(out=st[:], in_=st_in)
                bt = consts.tile([P, HT], f32)
                weng.dma_start(out=bt[:], in_=bt_in)

                # input stream, one FIFO ring: W early/interleaved so energy
                # is ready long before the enc tiles it gates
                wts = [None] * HT
                wgs = [None] * 2
                ets = [None] * HT

                def load_w(jt):
                    wt = wpool.tile([P, HT, P], f16)
                    nc.sync.dma_start(out=wt[:], in_=wtb[jt])
                    wts[jt] = wt

                def load_wg(g):
                    # sstat: 1 MB group of 4 W i-tiles [p, it, j]
                    wg = wpool.tile([P, 4, H], f16)
                    weng.dma_start(
                        out=wg[:],
                        in_=wit[4 * g:4 * (g + 1)].rearrange("t p j -> p t j"),
                    )
                    wgs[g] = wg

                def load_e(h0):
                    # one transfer covering h-tiles [h0, h0+ENC_CHUNK)
                    et = encpool.tile([P, ENC_CHUNK, BL, S], f16)
                    nc.sync.dma_start(
                        out=et[:],
                        in_=enc_t[h0:h0 + ENC_CHUNK].rearrange("t p b s -> p t b s"),
                    )
                    for k in range(ENC_CHUNK):
                        ets[h0 + k] = (et, k)

                eloads = [h0 for h0 in range(0, HT, ENC_CHUNK)]
                if sstat:
                    load_wg(0)
                    load_wg(1)
                    for h0 in eloads:
                        load_e(h0)
                else:
                    wloads = list(range(HT))
                    order = []
                    for h0 in eloads:
                        while wloads and len(order) % 3 != 2:
                            order.append(("w", wloads.pop(0)))
                        order.append(("e", h0))
                    for it_, idx in order:
                        (load_w if it_ == "w" else load_e)(idx)
                    for jt in wloads:
                        load_w(jt)

                # energy[j % 128, jt, b] = sum_i W[j,i] state[b,i] + bias[j]
                energy = consts.tile([P, HT, BL], f16)
                if sstat:
                    # energy_T[b, j] accumulated over i-tiles; state is the
                    # (2-column) stationary operand so LDWEIGHTS is ~free and
                    # W streams through the moving path at 1 col/cycle
                    et_ps = pe_pool.tile([BL, H], f32)
                    for it in range(HT):
                        for jh in range(2):
                            nc.tensor.matmul(
                                et_ps[:, jh * SCW:(jh + 1) * SCW],
                                st[:, it, :],                            # lhsT [i, b]
                                wgs[it // 4][:, it % 4, jh * SCW:(jh + 1) * SCW],
                                start=(it == 0),
                                stop=(it == HT - 1),
                            )
                    esb = consts.tile([BL, H], f16)
                    nc.scalar.activation(
                        out=esb[:], in_=et_ps[:],
                        func=mybir.ActivationFunctionType.Identity,
                        bias=0.0, scale=1.0,
                    )
                    id2 = consts.tile([BL, BL], f16)
                    weng.dma_start(out=id2[:], in_=id_in)
                    for jt in range(HT):
                        tp = tp_pool.tile([P, BL], f16)
                        nc.tensor.transpose(
                            tp[:], esb[:, jt * P:(jt + 1) * P], id2[:]
                        )
                        nc.vector.tensor_scalar_add(
                            out=energy[:, jt, :], in0=tp[:],
                            scalar1=bt[:, jt:jt + 1],
                        )
                else:
                    for jt in range(HT):
                        pe = pe_pool.tile([P, BL], f32)
                        for it in range(HT):
                            nc.tensor.matmul(
                                pe[:],
                                wts[jt][:, it, :],  # lhsT [i, j]
                                st[:, it, :],       # rhs  [i, b]
                                start=(it == 0),
                                stop=(it == HT - 1),
                            )
                        nc.vector.tensor_scalar_add(
                            out=energy[:, jt, :], in0=pe[:], scalar1=bt[:, jt:jt + 1]
                        )

                # scores psum: b=0 on partition 0, b=1 on partition 32; the
                # two batches run concurrently in separate 32-col groups
                ps = ps_pool.tile([33, S], f32)
                for ht in range(HT):
                    et, k = ets[ht]
                    for sc in range(SC):
                        for b in range(BL):
                            nc.tensor.matmul(
                                ps[32 * b:32 * b + 1, sc * SCW:(sc + 1) * SCW],
                                energy[:, ht, b:b + 1],                    # lhsT [h, 1]
                                et[:, k, b, sc * SCW:(sc + 1) * SCW],      # rhs [h, s]
                                start=(ht == 0),
                                stop=(ht == HT - 1),
                                tile_position=(0, 32 * b),
                            )

                # softmax over s (free dim); both batches (partitions 0 and
                # 32) processed jointly in one instruction per stage
                prob_sb = spool.tile([33, S], f32)
                nmax = spool.tile([33, 1], f32)
                ssum = spool.tile([33, 1], f32)
                rinv = spool.tile([33, 1], f32)
                nc.vector.reduce_max(
                    nmax[:], ps[:], axis=mybir.AxisListType.X, negate=True
                )
                nc.scalar.activation(
                    out=prob_sb[:],
                    in_=ps[:],
                    func=mybir.ActivationFunctionType.Exp,
                    bias=nmax[:],
                    scale=1.0,
                    accum_out=ssum[:],
                )
                nc.vector.reciprocal(rinv[:], ssum[:])
                nc.vector.tensor_scalar_mul(
                    out=prob_sb[:], in0=prob_sb[:], scalar1=rinv[:]
                )
                # output on the scalar ring: a tail store must never block
                # the next rep's input stream on the sync ring
                for b in range(BL):
                    nc.scalar.dma_start(
                        out=probs[b:b + 1, :], in_=prob_sb[32 * b:32 * b + 1, :]
                    )

            if dynamic and reps > 1:
                assert reps % UNROLL == 0, (reps, UNROLL)
                with tc.For_i(0, reps // UNROLL, 1):
                    for _u in range(UNROLL):
                        emit_rep()
            else:
                for _rep in range(reps):
                    emit_rep()

    nc.compile()
    return nc


def get_nc(reps=1, dynamic=False):
    key = ("nc", reps, dynamic, UNROLL, ENC_CHUNK, ENC_BUFS, W_BUFS, ENERGY_MODE,
           DMA_SPLIT)
    if key not in _cached:
        _cached[key] = _build_nc(reps, dynamic)
    return _cached[key]


def prep_in_maps(encoder_output, last_decoder_state, W, b):
    enc16 = np.asarray(encoder_output, dtype=np.float32).astype(np.float16)  # [S,B,H]
    state = np.asarray(last_decoder_state, dtype=np.float32)[0, 0]           # [B,H]
    W32 = np.asarray(W, dtype=np.float32)
    # wtb[jt, p, it, j] = W[jt*128+j, it*128+p]
    W16 = W32.astype(np.float16)
    wtb = np.ascontiguousarray(W16.reshape(HT, P, HT, P).transpose(0, 3, 2, 1))
    wit = np.ascontiguousarray(W16.T).reshape(HT, P, H)
    bias = np.asarray(b, dtype=np.float32)
    bt = np.ascontiguousarray(bias.reshape(HT, P).T)                         # [p, jt]
    in_maps = []
    for c in range(NCORES):
        b0 = BL * c
        ec = enc16[:, b0:b0 + BL, :]                                         # [S,BL,H]
        enc_t = np.ascontiguousarray(ec.transpose(2, 1, 0)).reshape(HT, P, BL, S)
        stc = state[b0:b0 + BL, :].astype(np.float16)                        # [BL,H]
        st = np.ascontiguousarray(stc.reshape(BL, HT, P).transpose(2, 1, 0))
        in_maps.append({"enc_t": enc_t, "wtb": wtb, "wit": wit,
                        "st_in": st, "bt_in": bt,
                        "id_in": np.eye(BL, dtype=np.float16)})
    return in_maps


def assemble(results):
    out = np.empty((S, B), np.float32)
    for c in range(NCORES):
        out[:, BL * c:BL * (c + 1)] = results[c]["probs"].T
    return out[None, None]


def kernel(encoder_output, last_decoder_state, W, b):
    from concourse.bass_utils import run_bass_kernel_spmd

    nc = get_nc()
    in_maps = prep_in_maps(encoder_output, last_decoder_state, W, b)
    res = run_bass_kernel_spmd(nc, in_maps, core_ids=list(range(NCORES)))
    return assemble(res.results)
